# revision 1
# baseline (speedup 1.0000x reference)
"""Trainium2 Bass kernel for nn_Demolition_splitweight_Conv2d.

Computation (per batch element b, one NeuronCore each):
    out[o, p] = (1/(127*Q)) * sum_k wvec[k] * sum_c round(Q*(conv3x3(x[c]; w[k,c,o]) + b[k,c,o]))
with Q = 12.5, wvec = [-128, 1, 2, 4, 8, 16, 32, 64].

Key trick: the per-(k,c) round-to-nearest-even happens INSIDE the TensorEngine
matmul accumulation via the fp32 magic-number trick: bf16 matmuls on TRN2
accumulate strictly row-sequentially in fp32, so a contraction-row layout of
    [27 split-product tap rows, bias_hi, bias_lo, +M, -M]   (M = 1.5*2^23)
per channel c yields exactly round(Q*y_c) added into PSUM — conv +
per-channel quantization + channel-sum is pure matmul work.

Precision: 3-term Dekker split (w_hi*x_hi + w_hi*x_lo + w_lo*x_hi), bf16
inputs / fp32 accumulation.

Layout: data-parallel over batch (8 cores). Per core the host pre-builds a
"replicated tap image" REP [128, 8*PSZA]: partition (cl*32 + t) holds the
zero-padded channel image of c = 4q + cl (q indexes the free-dim block),
pre-shifted by tap t's (dy, dx) — so every conv matmul's moving operand is a
plain contiguous AP slice and the whole input side uploads in a handful of
large DMAs.
"""

import numpy as np
import ml_dtypes

import concourse.bass as bass
import concourse.mybir as mybir
from concourse.ap import AP
from concourse.tile import TileContext
from concourse.bass_utils import run_bass_kernel_spmd

# problem dims (hardcoded per the task contract)
B, C, OUT, H, W = 8, 32, 32, 64, 64
KBITS = 8
Q = 12.5
MAGIC = 12582912.0  # 1.5 * 2^23
WVEC = np.array([-128, 1, 2, 4, 8, 16, 32, 64], np.float32)
SCALE = float(1.0 / (127.0 * Q))

PW = 66            # padded width  (1 + 64 + 1)
PH = 66            # padded height (1 + 64 + 1)
PSZ = PH * PW      # 4356
PSZA = 4232        # q-block row length: max window col 4224 (+ pad)
ROW = 8 * PSZA     # REP free width: 8 chunk blocks side by side
NROW = 7           # image rows per pixel block
NPIX = NROW * PW   # 462 = matmul moving free dim; 462*4B < 2KB PSUM bank
NPB = 10           # pixel blocks (rows 0..62 in blocks of 7; block 9 covers row 63)

# upload segments (REP free-dim ranges, aligned to pixel blocks)
SEGS = [(i * NPIX, (i + 1) * NPIX) for i in range(9)] + [(9 * NPIX, PSZA)]
# per-block moving free dim: 9 blocks of 7 rows + 1 block covering row 63 only
NPIX_PB = [NPIX] * 9 + [PW]

BF = mybir.dt.bfloat16
F32 = mybir.dt.float32

_cache = {}


def _bf16(a):
    return np.asarray(a, np.float32).astype(ml_dtypes.bfloat16)


def _prep_weights(weight, bias):
    """Build lhsT [128, 16*128] and gmat [128, 64] host arrays (bf16)."""
    qw = (Q * weight.astype(np.float32)).reshape(KBITS, C, OUT, 9)  # [k,c,o,t]
    whi = _bf16(qw).astype(np.float32)
    wlo = _bf16(qw - whi).astype(np.float32)
    qb = (Q * bias.astype(np.float32)).reshape(KBITS, C, OUT)
    bhi = _bf16(qb).astype(np.float32)
    blo = _bf16(qb - bhi).astype(np.float32)

    # mat[p, j]: p = cl*32 + t (c = 4q + cl), j = k*16 + ol, o = mblk*16 + ol
    k_of = np.arange(128) // 16
    ol_of = np.arange(128) % 16
    lhsT = np.zeros((128, 16, 128), np.float32)
    for mblk in range(2):
        o_of = mblk * 16 + ol_of
        for q in range(8):
            mat = np.zeros((128, 128), np.float32)
            for cl in range(4):
                c = q * 4 + cl
                r0 = cl * 32
                mat[r0 + 0 : r0 + 9, :] = whi[k_of, c, o_of].T
                mat[r0 + 9 : r0 + 18, :] = whi[k_of, c, o_of].T
                mat[r0 + 18 : r0 + 27, :] = wlo[k_of, c, o_of].T
                mat[r0 + 27, :] = bhi[k_of, c, o_of]
                mat[r0 + 28, :] = blo[k_of, c, o_of]
                mat[r0 + 29, :] = MAGIC
                mat[r0 + 30, :] = -MAGIC
                # r0+31 stays 0 (pad row; rhs content is ones)
            lhsT[:, mblk * 8 + q, :] = mat
    lhsT_bf = _bf16(lhsT.reshape(128, 16 * 128))

    gmat = np.zeros((128, 64), np.float32)
    j = np.arange(128)
    gmat[j, ol_of] = WVEC[k_of]             # mblk0 -> out cols 0..15
    gmat[j, 32 + 16 + ol_of] = WVEC[k_of]   # mblk1 -> out cols 16..31
    return lhsT_bf, _bf16(gmat)


def _build_xrep(x):
    """Host-side REP construction: [B, 128, 8*PSZA] bf16.

    partition p = cl*32 + t, free block q; channel c = 4q + cl:
      t 0..8   : padded bf16(x_hi)[c] shifted by (dy, dx) = (t//3, t%3)
      t 9..17  : padded x_lo shifted
      t 18..26 : padded x_hi shifted (dup for the w_lo rows)
      t 27..31 : ones
    """
    xh = _bf16(x)
    xl = _bf16(x - xh.astype(np.float32))

    PADW = PSZA + 140
    def pad(img):  # [B, C, H, W] bf16 -> [B, C, PADW]
        p = np.zeros((B, C, PADW), ml_dtypes.bfloat16)
        p[:, :, :PSZ].reshape(B, C, PH, PW)[:, :, 1 : H + 1, 1 : W + 1] = img
        return p

    ph, pl = pad(xh), pad(xl)
    offs = [(t // 3) * PW + (t % 3) for t in range(9)]
    # shifted[s, b, c, j] = pad[b, c, offs[s] + j]
    sh_h = np.stack([ph[:, :, o : o + PSZA] for o in offs], axis=0)
    sh_l = np.stack([pl[:, :, o : o + PSZA] for o in offs], axis=0)

    xrep = np.zeros((B, 4, 32, 8, PSZA), ml_dtypes.bfloat16)
    cidx = (4 * np.arange(8)[None, :] + np.arange(4)[:, None])  # [cl, q] -> c
    # [9, B, cl, q, PSZ] views
    hi = sh_h[:, :, cidx, :].transpose(1, 2, 0, 3, 4)  # [B, cl, 9, q, PSZ]
    lo = sh_l[:, :, cidx, :].transpose(1, 2, 0, 3, 4)
    xrep[:, :, 0:9, :, :] = hi
    xrep[:, :, 9:18, :, :] = lo
    xrep[:, :, 18:27, :, :] = hi
    xrep[:, :, 27:32, :, :] = ml_dtypes.bfloat16(1.0)
    return xrep.reshape(B, 128, ROW)


def _split_multiwaits(nc):
    """This container's walrus allows one sync-wait per instruction; move
    extras onto preceding same-engine NoOps."""
    for bb in nc.main_func.blocks:
        insts = bb.instructions
        i = 0
        while i < len(insts):
            ins = insts[i]
            si = getattr(ins, "sync_info", None)
            if si is not None and si.on_wait is not None and len(si.on_wait) > 1:
                waits = list(si.on_wait)
                nops = []
                for j, w in enumerate(waits[:-1]):
                    nop = mybir.InstNoOp(name=f"{ins.name}-wsplit{j}", ins=[], outs=[])
                    nop.engine = ins.engine
                    nop.sync_info = mybir.SyncInfo(on_wait=[w], on_update=[])
                    nops.append(nop)
                si.on_wait = [waits[-1]]
                ins.sync_info = si
                for j, nop in enumerate(nops):
                    insts.insert(i + j, nop)
                i += len(nops)
            i += 1


def _build_nc():
    nc = bass.Bass()
    xrep_d = nc.dram_tensor("xrep", [128, ROW], BF, kind="ExternalInput")
    wc_d = nc.dram_tensor("wconst", [128, 16 * 128 + 64], BF, kind="ExternalInput")
    out_d = nc.dram_tensor("out", [OUT, H * W], F32, kind="ExternalOutput")

    with TileContext(nc) as tc:
        with (
            tc.tile_pool(name="const", bufs=1) as cpool,
            tc.tile_pool(name="work", bufs=6) as wpool,
            tc.tile_pool(name="outp", bufs=4) as opool,
            tc.tile_pool(name="psP", bufs=5, space="PSUM") as psP,
            tc.tile_pool(name="psR", bufs=2, space="PSUM") as psR,
        ):
            wconst = cpool.tile([128, 16 * 128 + 64], BF, tag="wconst")
            lhsT = wconst[:, 0 : 16 * 128]
            gmat = wconst[:, 16 * 128 : 16 * 128 + 64]
            nc.sync.dma_start(out=wconst[:, 0:256], in_=wc_d[:, 0:256])

            rep = cpool.tile([128, ROW], BF, tag="rep")

            def seg_dma(s0, s1):
                dst = AP(tensor=rep.tensor, offset=rep.offset + s0,
                         ap=[[ROW, 128], [PSZA, 8], [1, s1 - s0]])
                src = AP(tensor=xrep_d, offset=s0,
                         ap=[[ROW, 128], [PSZA, 8], [1, s1 - s0]])
                nc.sync.dma_start(out=dst, in_=src)

            act_warm = opool.tile([32, NPIX], F32, tag="osb", name="actwarm")
            nc.scalar.copy(act_warm[:, 0:64], gmat[0:32, :])
            warm_ps = psP.tile([128, NPIX], F32, tag="P", name="warmps")
            for wi in range(12):
                nc.tensor.matmul(warm_ps[:, :256], lhsT[:, 0:128],
                                 lhsT[:, 0:256], start=(wi == 0),
                                 stop=(wi == 11))
            seg_dma(*SEGS[0])
            nc.sync.dma_start(out=wconst[:, 256:], in_=wc_d[:, 256:])
            for s0, s1 in SEGS[1:]:
                seg_dma(s0, s1)

            for pb in range(NPB):
                base = pb * NPIX
                n = NPIX_PB[pb]
                a_tiles = []
                P01 = [psP.tile([128, NPIX], F32, tag="P", name=f"P{pb}_{i}") for i in range(2)]
                for q in range(8):
                    rhs = rep[:, q * PSZA + base : q * PSZA + base + n]
                    for m in range(2):
                        w_ap = lhsT[:, (m * 8 + q) * 128 : (m * 8 + q + 1) * 128]
                        nc.tensor.matmul(P01[m][:, :n], w_ap, rhs,
                                         start=(q == 0), stop=(q == 7))
                for m in range(2):
                    A = wpool.tile([128, NPIX], BF, tag="A")
                    if m == 0:
                        nc.vector.tensor_copy(A[:, :n], P01[m][:, :n])
                    else:
                        nc.scalar.copy(A[:, :n], P01[m][:, :n])
                    a_tiles.append(A)
                R = psR.tile([32, NPIX], F32, tag="R")
                nc.tensor.matmul(R[:, :n], gmat[:, 0:32], a_tiles[0][:, :n],
                                 start=True, stop=False)
                nc.tensor.matmul(R[:, :n], gmat[:, 32:64], a_tiles[1][:, :n],
                                 start=False, stop=True)
                osb = opool.tile([32, NPIX], F32, tag="osb")
                nc.scalar.mul(osb[:, :n], R[:, :n], SCALE)

                nr = NROW if pb < NPB - 1 else H - (NPB - 1) * NROW
                dst = AP(tensor=out_d, offset=pb * NROW * W,
                         ap=[[H * W, OUT], [W, nr], [1, W]])
                s = AP(tensor=osb.tensor, offset=osb.offset,
                       ap=[[NPIX, 32], [PW, nr], [1, W]])
                nc.sync.dma_start(out=dst, in_=s)

    _split_multiwaits(nc)
    return nc


def kernel(x, weight, bias):
    x = np.asarray(x, np.float32)
    weight = np.asarray(weight, np.float32)
    bias = np.asarray(bias, np.float32)

    xrep = _build_xrep(x)
    lhsT, gmat = _prep_weights(weight, bias)
    wconst = np.concatenate([lhsT, gmat], axis=1)

    if "nc" not in _cache:
        _cache["nc"] = _build_nc()
    nc = _cache["nc"]

    in_maps = [{"xrep": xrep[b], "wconst": wconst} for b in range(B)]
    res = run_bass_kernel_spmd(nc, in_maps, core_ids=list(range(B)))
    out = np.stack([r["out"] for r in res.results])
    return out.reshape(B, OUT, H, W).astype(np.float32)



# revision 10
# speedup vs baseline: 1.6423x; 1.6423x over previous
"""Trainium2 Bass kernel for nn_Demolition_splitweight_Conv2d.

Computation (per batch element b, one NeuronCore each):
    out[o, p] = (1/(127*Q)) * sum_k wvec[k] * sum_c round(Q*(conv3x3(x[c]; w[k,c,o]) + b[k,c,o]))
with Q = 12.5, wvec = [-128, 1, 2, 4, 8, 16, 32, 64].

Scheme: fp16 single-term products + fp32 magic-number rounding inside the
TensorEngine accumulation. fp16's 11-bit significand makes Dekker splitting
unnecessary (measured rel err ~9e-3 vs the 2e-2 gate), so each input channel
needs only 12 contraction rows:
    [9 tap rows, bias row (rhs=ones), +M row, -M row]   (M = 1.5*2^23 = 3072*4096)

HW probe result (probe_seg32.py): the PE accumulates a matmul's contraction
rows sequentially WITHIN 32-row segments, then combines segment totals
atomically (and likewise chained matmuls combine atomically through PSUM).
A magic-rounding unit therefore must sit entirely inside one 32-row segment:
2 channels per segment (24 rows + 8 zero pad), 8 channels per 128-row pass,
4 passes x 2 output halves (m in {0,1}: out cols (k,ol) = 8 bits x 16 chans)
per pixel block.

Layout: data-parallel over batch (8 cores). Host uploads a tap-replicated
fp16 image REP [128, 4*4232]: column block g holds pass g's rhs rows; the
row for (channel c, tap j) is c's padded image pre-shifted by tap j; bias
rows are ones, magic rows 4096.0 (lhsT carries +-3072), pad rows zero.

Stage 2 (bit recombination via wvec*SCALE matrix; PSUM -> fp16 A copies on
vector/scalar, packing out the 2 pad cols per image row) interleaves in PE
program order after the next block's stage 1 so the PE never stalls.
"""

import numpy as np

import concourse.bass as bass
import concourse.mybir as mybir
from concourse.ap import AP
from concourse.tile import TileContext
from concourse.bass_utils import run_bass_kernel_spmd

# problem dims (hardcoded per the task contract)
B, C, OUT, H, W = 8, 32, 32, 64, 64
KBITS = 8
Q = 12.5
WVEC = np.array([-128, 1, 2, 4, 8, 16, 32, 64], np.float32)
SCALE = float(1.0 / (127.0 * Q))

PW = 66            # padded width  (1 + 64 + 1)
PH = 66            # padded height
PSZ = PH * PW      # 4356
PSZA = 4232        # pass-block row length (max window 4224 + slack)
PADW = PSZA + 134  # host pad buffer: max tap offset 134
NG = 4             # stage-1 passes: 4 x (4 segments x 2 channels)
RPC = 12           # contraction rows per channel
NROW = 7           # image rows per pixel block
NPIX = NROW * PW   # 462 = stage-1 moving free dim; 462*4B < 2KB PSUM bank
NPB = 10           # pixel blocks: rows 0..62 in blocks of 7; block 9 = row 63

SEGS = [(i * NPIX, (i + 1) * NPIX) for i in range(9)] + [(9 * NPIX, PSZA)]
NPIX_PB = [NPIX] * 9 + [PW]

NWARM = 32          # PE p-state warmup matmuls (128 cols each)
WCW = 2 * NG * 128 + 64  # weight cols: 8 stage-1 lhsT blocks + gmat
BOOTW = WCW + NG * NPIX  # boot tensor: weights + pixel block 0
BOOT1W = WCW + 2 * NPIX  # first boot DMA: weights + block 0 passes 0-1

F16 = mybir.dt.float16
F32 = mybir.dt.float32

_cache = {}


def _f16(a):
    return np.asarray(a, np.float32).astype(np.float16)


def _row_of(c, j):
    """Contraction row (pass g, partition w) for channel c, intra row j."""
    g, r = divmod(c, 8)
    s, t = divmod(r, 2)
    return g, s * 32 + t * RPC + j


def _prep_weights(weight, bias):
    """Weight cols [128, WCW] fp16: 8 stage-1 lhsT blocks (g,m) + gmat."""
    qw = _f16(Q * weight.astype(np.float32)).reshape(KBITS, C, 2, 16, 9)
    qb = _f16(Q * bias.astype(np.float32)).reshape(KBITS, C, 2, 16)

    Wt = np.zeros((NG, 128, 2, 128), np.float16)
    for c in range(C):
        for j in range(9):
            g, w = _row_of(c, j)
            # qw[k,c,m,ol] -> [m, (k*16+ol)]
            Wt[g, w] = qw[:, c, :, :, j].transpose(1, 0, 2).reshape(2, 128)
        g, w = _row_of(c, 9)
        Wt[g, w] = qb[:, c].transpose(1, 0, 2).reshape(2, 128)
        g, w = _row_of(c, 10)
        Wt[g, w] = np.float16(3072.0)
        g, w = _row_of(c, 11)
        Wt[g, w] = np.float16(-3072.0)

    wc = np.zeros((128, WCW), np.float16)
    for g in range(NG):
        for m in range(2):
            wc[:, (g * 2 + m) * 128:(g * 2 + m + 1) * 128] = Wt[g, :, m, :]

    j = np.arange(128)
    k_of, ol_of = j // 16, j % 16
    gmat = np.zeros((128, 64), np.float32)
    gmat[j, ol_of] = WVEC[k_of] * SCALE           # A0 -> out rows 0..15
    gmat[j, 32 + 16 + ol_of] = WVEC[k_of] * SCALE  # A1 -> out rows 16..31
    wc[:, 2 * NG * 128:] = _f16(gmat)
    return wc


def _build_xrep(x):
    """Host REP construction: [B, 128, NG*PSZA] fp16 (see module docstring)."""
    xh = _f16(x)
    xpad = np.zeros((B, C, PADW), np.float16)
    xpad[:, :, :PSZ].reshape(B, C, PH, PW)[:, :, 1:H + 1, 1:W + 1] = xh

    rep = np.zeros((B, NG, 128, PSZA), np.float16)
    for j in range(9):
        off = (j // 3) * PW + (j % 3)
        for c in range(C):
            g, w = _row_of(c, j)
            rep[:, g, w, :] = xpad[:, c, off:off + PSZA]
    for c in range(C):
        g, w = _row_of(c, 9)
        rep[:, g, w, :] = np.float16(1.0)
        for j in (10, 11):
            g, w = _row_of(c, j)
            rep[:, g, w, :] = np.float16(4096.0)
    return rep.transpose(0, 2, 1, 3).reshape(B, 128, NG * PSZA)


def _split_multiwaits(nc):
    """This container's walrus allows one sync-wait per instruction; move
    extras onto preceding same-engine NoOps."""
    for bb in nc.main_func.blocks:
        insts = bb.instructions
        i = 0
        while i < len(insts):
            ins = insts[i]
            si = getattr(ins, "sync_info", None)
            if si is not None and si.on_wait is not None and len(si.on_wait) > 1:
                waits = list(si.on_wait)
                nops = []
                for j, w in enumerate(waits[:-1]):
                    nop = mybir.InstNoOp(name=f"{ins.name}-wsplit{j}", ins=[], outs=[])
                    nop.engine = ins.engine
                    nop.sync_info = mybir.SyncInfo(on_wait=[w], on_update=[])
                    nops.append(nop)
                si.on_wait = [waits[-1]]
                ins.sync_info = si
                for j, nop in enumerate(nops):
                    insts.insert(i + j, nop)
                i += len(nops)
            i += 1


def _build_nc():
    nc = bass.Bass()
    boot_d = nc.dram_tensor("boot", [128, BOOTW], F16, kind="ExternalInput")
    xrep_d = nc.dram_tensor("xrep", [128, NG * PSZA], F16, kind="ExternalInput")
    out_d = nc.dram_tensor("out", [OUT, H * W], F32, kind="ExternalOutput")

    with TileContext(nc) as tc:
        with (
            tc.tile_pool(name="const", bufs=1) as cpool,
            tc.tile_pool(name="work", bufs=4) as wpool,
            tc.tile_pool(name="outp", bufs=2) as opool,
            tc.tile_pool(name="psP", bufs=4, space="PSUM") as psP,
            tc.tile_pool(name="psR", bufs=3, space="PSUM") as psR,
        ):
            # warmup: PE p-state ramp on a zero tile (memset on the
            # otherwise-idle Pool engine for the shortest dep latency)
            warm = cpool.tile([128, 128], F16, tag="warm")
            nc.gpsimd.memset(warm[:, :], 0.0)
            warm_ps = psP.tile([128, NPIX], F32, tag="P", name="warmps")
            for wi in range(NWARM):
                nc.tensor.matmul(warm_ps[:, :128], warm[:, :], warm[:, :],
                                 start=True, stop=True)

            boot = cpool.tile([128, BOOTW], F16, tag="boot")
            # split boot upload: weights + block-0 passes 0-1 first so
            # stage 1 can begin one DMA-transfer earlier
            nc.sync.dma_start(out=boot[:, :BOOT1W], in_=boot_d[:, :BOOT1W])
            nc.sync.dma_start(out=boot[:, BOOT1W:], in_=boot_d[:, BOOT1W:])
            wconst = boot[:, 0:WCW]

            rep = cpool.tile([128, NG * PSZA], F16, tag="rep")

            def seg_dma(s0, s1):
                dst = AP(tensor=rep.tensor, offset=rep.offset + s0,
                         ap=[[NG * PSZA, 128], [PSZA, NG], [1, s1 - s0]])
                src = AP(tensor=xrep_d, offset=s0,
                         ap=[[NG * PSZA, 128], [PSZA, NG], [1, s1 - s0]])
                nc.sync.dma_start(out=dst, in_=src)

            for s0, s1 in SEGS[1:]:
                seg_dma(s0, s1)

            def stage1(pb):
                base, npad = pb * NPIX, NPIX_PB[pb]
                nr = NROW if pb < NPB - 1 else 1
                n = nr * W
                P = [psP.tile([128, NPIX], F32, tag="P", name=f"P{pb}_{m}")
                     for m in range(2)]
                for g in range(NG):
                    if pb == 0:
                        rhs = boot[:, WCW + g * NPIX: WCW + g * NPIX + npad]
                    else:
                        rhs = rep[:, g * PSZA + base: g * PSZA + base + npad]
                    for m in range(2):
                        lt = wconst[:, (g * 2 + m) * 128:(g * 2 + m + 1) * 128]
                        nc.tensor.matmul(P[m][:, :npad], lt, rhs,
                                         start=(g == 0), stop=(g == NG - 1))
                # PSUM -> fp16, packing out the 2 pad cols per image row so
                # stage 2 and the output DMA run on dense 64-col rows
                A = [wpool.tile([128, NROW * W], F16, tag="A", name=f"A{pb}_{m}")
                     for m in range(2)]
                pin = [AP(tensor=P[m].tensor, offset=P[m].offset,
                          ap=[[NPIX, 128], [PW, nr], [1, W]]) for m in range(2)]
                # last (tiny) block: both copies on DVE — the scalar engine
                # is still busy with the previous block's copy
                nc.vector.tensor_copy(A[0][:, :n], pin[0])
                if pb == NPB - 1:
                    nc.vector.tensor_copy(A[1][:, :n], pin[1])
                else:
                    nc.scalar.copy(A[1][:, :n], pin[1])
                return A

            osb = None

            def stage2(pb, A):
                nonlocal osb
                nr = NROW if pb < NPB - 1 else 1
                n = nr * W
                R = psR.tile([32, NROW * W], F32, tag="R", name=f"R{pb}")
                gm0 = wconst[:, 2 * NG * 128:2 * NG * 128 + 32]
                gm1 = wconst[:, 2 * NG * 128 + 32:2 * NG * 128 + 64]
                nc.tensor.matmul(R[:, :n], gm0, A[0][:, :n],
                                 start=True, stop=False)
                nc.tensor.matmul(R[:, :n], gm1, A[1][:, :n],
                                 start=False, stop=True)
                if pb % 2 == 0:
                    osb = opool.tile([32, 2 * NROW * W], F32, tag="osb",
                                     name=f"osb{pb}")
                half = (pb % 2) * NROW * W
                out_ap = AP(tensor=osb.tensor, offset=osb.offset + half,
                            ap=[[2 * NROW * W, 32], [1, n]])
                # alternate osb copies between engines; keep the final pair's
                # copies off the congested scalar queue
                if pb % 2 == 0:
                    nc.vector.tensor_copy(out_ap, R[:, :n])
                else:
                    nc.scalar.copy(out_ap, R[:, :n])
                if pb % 2 == 1:
                    nrows = NROW + nr
                    dst = AP(tensor=out_d, offset=(pb - 1) * NROW * W,
                             ap=[[H * W, OUT], [1, nrows * W]])
                    src = AP(tensor=osb.tensor, offset=osb.offset,
                             ap=[[2 * NROW * W, 32], [1, nrows * W]])
                    nc.sync.dma_start(out=dst, in_=src)

            prevA = None
            for pb in range(NPB):
                A = stage1(pb)
                if prevA is not None:
                    stage2(pb - 1, prevA)
                prevA = A
            stage2(NPB - 1, prevA)

    _split_multiwaits(nc)
    return nc


def kernel(x, weight, bias):
    x = np.asarray(x, np.float32)
    weight = np.asarray(weight, np.float32)
    bias = np.asarray(bias, np.float32)

    xrep = _build_xrep(x)
    wc = _prep_weights(weight, bias)

    if "nc" not in _cache:
        _cache["nc"] = _build_nc()
    nc = _cache["nc"]

    seg0 = xrep.reshape(B, 128, NG, PSZA)[:, :, :, :NPIX].reshape(B, 128, NG * NPIX)
    in_maps = [
        {"boot": np.concatenate([wc, seg0[b]], axis=1), "xrep": xrep[b]}
        for b in range(B)
    ]
    res = run_bass_kernel_spmd(nc, in_maps, core_ids=list(range(B)))
    out = np.stack([r["out"] for r in res.results])
    return out.reshape(B, OUT, H, W).astype(np.float32)


# revision 12
# speedup vs baseline: 1.6705x; 1.0172x over previous
"""Trainium2 Bass kernel for nn_Demolition_splitweight_Conv2d.

Computation (per batch element b, one NeuronCore each):
    out[o, p] = (1/(127*Q)) * sum_k wvec[k] * sum_c round(Q*(conv3x3(x[c]; w[k,c,o]) + b[k,c,o]))
with Q = 12.5, wvec = [-128, 1, 2, 4, 8, 16, 32, 64].

Scheme: fp16 single-term products + fp32 magic-number rounding inside the
TensorEngine accumulation. fp16's 11-bit significand makes Dekker splitting
unnecessary (measured rel err ~9e-3 vs the 2e-2 gate), so each input channel
needs only 12 contraction rows:
    [9 tap rows, bias row (rhs=ones), +M row, -M row]   (M = 1.5*2^23 = 3072*4096)

HW probe result (probe_seg32.py): the PE accumulates a matmul's contraction
rows sequentially WITHIN 32-row segments, then combines segment totals
atomically (and likewise chained matmuls combine atomically through PSUM).
A magic-rounding unit therefore must sit entirely inside one 32-row segment:
2 channels per segment (24 rows + 8 zero pad), 8 channels per pass, 4 passes
x 2 output halves (m in {0,1}: out cols (k,ol) = 8 bits x 16 channels) per
pixel block. The 4th segment needs no pad: contraction K = 120.

Layout: data-parallel over batch (8 cores). The host pre-shifts each
channel's zero-padded image by each tap offset and interleaves bias-ones /
magic-4096 / pad-zero rows, sliced per pixel block: one SBUF tile per block
[120, 4 passes x 462 cols], uploaded as one contiguous full-rate DMA each.
Per-block tiles keep the tile-framework's whole-tile dependency exact, which
lets stage-1 matmuls use a 3D rhs AP that skips the 2 pad columns per image
row (moving dim 448 = 7 rows x 64 px, PSUM bank = 1792B).

Stage 2 (bit recombination via the wvec*SCALE matrix; PSUM -> fp16 A copies
on vector/scalar) interleaves in PE program order after the NEXT block's
stage 1 so the PE never stalls on copy latency. A boot DMA carries weights +
pixel-block 0 so real matmuls start one DMA-latency after launch; dep-free
warmup matmuls ramp the PE p-state meanwhile.
"""

import numpy as np

import concourse.bass as bass
import concourse.mybir as mybir
from concourse.ap import AP
from concourse.tile import TileContext
from concourse.bass_utils import run_bass_kernel_spmd

# problem dims (hardcoded per the task contract)
B, C, OUT, H, W = 8, 32, 32, 64, 64
KBITS = 8
Q = 12.5
WVEC = np.array([-128, 1, 2, 4, 8, 16, 32, 64], np.float32)
SCALE = float(1.0 / (127.0 * Q))

PW = 66            # padded width  (1 + 64 + 1)
PH = 66            # padded height
PSZ = PH * PW      # 4356
NG = 4             # stage-1 passes: 4 x (4 segments x 2 channels)
KC = 120           # contraction rows per pass (last segment unpadded)
RPC = 12           # contraction rows per channel
NROW = 7           # image rows per pixel block
NPIX = NROW * PW   # 462 = per-block source window width
NDEN = NROW * W    # 448 = dense moving dim (pad cols skipped)
NPB = 10           # pixel blocks: rows 0..62 in blocks of 7; block 9 = row 63
BLKW = NG * NPIX   # per-block tile width (block 9: NG * PW)
PADW = 10 * NPIX + 134  # host pad buffer width

NPIX_PB = [NPIX] * 9 + [PW]

NWARM = 32          # PE p-state warmup matmuls (128 cols each)
WCW = 2 * NG * 128 + 64  # weight cols: 8 stage-1 lhsT blocks + gmat
BOOTW = WCW + BLKW       # boot tensor: weights + pixel block 0
BOOT1W = WCW + 2 * NPIX  # first boot DMA: weights + block 0 passes 0-1
XREPW = 8 * BLKW + NG * PW  # xrep dram: blocks 1..8 full + block 9 narrow

F16 = mybir.dt.float16
F32 = mybir.dt.float32

_cache = {}


def _f16(a):
    return np.asarray(a, np.float32).astype(np.float16)


def _row_of(c, j):
    """Contraction row (pass g, partition w) for channel c, intra row j."""
    g, r = divmod(c, 8)
    s, t = divmod(r, 2)
    return g, s * 32 + t * RPC + j


def _prep_weights(weight, bias):
    """Weight cols [128, WCW] fp16: 8 stage-1 lhsT blocks (g,m) + gmat."""
    qw = _f16(Q * weight.astype(np.float32)).reshape(KBITS, C, 2, 16, 9)
    qb = _f16(Q * bias.astype(np.float32)).reshape(KBITS, C, 2, 16)

    Wt = np.zeros((NG, KC, 2, 128), np.float16)
    for c in range(C):
        for j in range(9):
            g, w = _row_of(c, j)
            # qw[k,c,m,ol] -> [m, (k*16+ol)]
            Wt[g, w] = qw[:, c, :, :, j].transpose(1, 0, 2).reshape(2, 128)
        g, w = _row_of(c, 9)
        Wt[g, w] = qb[:, c].transpose(1, 0, 2).reshape(2, 128)
        g, w = _row_of(c, 10)
        Wt[g, w] = np.float16(3072.0)
        g, w = _row_of(c, 11)
        Wt[g, w] = np.float16(-3072.0)

    wc = np.zeros((128, WCW), np.float16)
    for g in range(NG):
        for m in range(2):
            wc[:KC, (g * 2 + m) * 128:(g * 2 + m + 1) * 128] = Wt[g, :, m, :]

    j = np.arange(128)
    k_of, ol_of = j // 16, j % 16
    gmat = np.zeros((128, 64), np.float32)
    gmat[j, ol_of] = WVEC[k_of] * SCALE           # A0 -> out rows 0..15
    gmat[j, 32 + 16 + ol_of] = WVEC[k_of] * SCALE  # A1 -> out rows 16..31
    wc[:, 2 * NG * 128:] = _f16(gmat)
    return wc


def _build_xrep(x):
    """Host REP: [B, KC, 10 blocks, NG, block cols] fp16, block-sliced."""
    xh = _f16(x)
    xpad = np.zeros((B, C, PADW), np.float16)
    xpad[:, :, :PSZ].reshape(B, C, PH, PW)[:, :, 1:H + 1, 1:W + 1] = xh

    rep = np.zeros((B, NG, KC, 10 * NPIX), np.float16)
    for j in range(9):
        off = (j // 3) * PW + (j % 3)
        for c in range(C):
            g, w = _row_of(c, j)
            rep[:, g, w, :] = xpad[:, c, off:off + 10 * NPIX]
    for c in range(C):
        g, w = _row_of(c, 9)
        rep[:, g, w, :] = np.float16(1.0)
        for j in (10, 11):
            g, w = _row_of(c, j)
            rep[:, g, w, :] = np.float16(4096.0)
    # -> [B, KC, block, NG, NPIX], block 9 truncated to PW cols
    blocks = rep.reshape(B, NG, KC, 10, NPIX).transpose(0, 2, 3, 1, 4)
    full = blocks[:, :, :9].reshape(B, KC, 9 * BLKW)
    tail = blocks[:, :, 9, :, :PW].reshape(B, KC, NG * PW)
    return np.concatenate([full, tail], axis=2)  # [B, KC, XREPW]


def _split_multiwaits(nc):
    """This container's walrus allows one sync-wait per instruction; move
    extras onto preceding same-engine NoOps."""
    for bb in nc.main_func.blocks:
        insts = bb.instructions
        i = 0
        while i < len(insts):
            ins = insts[i]
            si = getattr(ins, "sync_info", None)
            if si is not None and si.on_wait is not None and len(si.on_wait) > 1:
                waits = list(si.on_wait)
                nops = []
                for j, w in enumerate(waits[:-1]):
                    nop = mybir.InstNoOp(name=f"{ins.name}-wsplit{j}", ins=[], outs=[])
                    nop.engine = ins.engine
                    nop.sync_info = mybir.SyncInfo(on_wait=[w], on_update=[])
                    nops.append(nop)
                si.on_wait = [waits[-1]]
                ins.sync_info = si
                for j, nop in enumerate(nops):
                    insts.insert(i + j, nop)
                i += len(nops)
            i += 1


def _build_nc():
    nc = bass.Bass()
    boot_d = nc.dram_tensor("boot", [128, BOOTW], F16, kind="ExternalInput")
    xrep_d = nc.dram_tensor("xrep", [KC, XREPW], F16, kind="ExternalInput")
    out_d = nc.dram_tensor("out", [OUT, H * W], F32, kind="ExternalOutput")

    with TileContext(nc) as tc:
        with (
            tc.tile_pool(name="const", bufs=1) as cpool,
            tc.tile_pool(name="blk", bufs=9) as bpool,
            tc.tile_pool(name="work", bufs=4) as wpool,
            tc.tile_pool(name="outp", bufs=2) as opool,
            tc.tile_pool(name="psP", bufs=4, space="PSUM") as psP,
            tc.tile_pool(name="psR", bufs=3, space="PSUM") as psR,
        ):
            # warmup: PE p-state ramp on a zero tile (memset on the
            # otherwise-idle Pool engine for the shortest dep latency)
            warm = cpool.tile([128, 128], F16, tag="warm")
            nc.gpsimd.memset(warm[:, :], 0.0)
            warm_ps = psP.tile([128, NDEN], F32, tag="P", name="warmps")
            for wi in range(NWARM):
                nc.tensor.matmul(warm_ps[:, :128], warm[:, :], warm[:, :],
                                 start=True, stop=True)

            boot = cpool.tile([128, BOOTW], F16, tag="boot")
            # split boot upload: weights + block-0 passes 0-1 first so
            # stage 1 can begin one DMA-transfer earlier
            nc.sync.dma_start(out=boot[:, :BOOT1W], in_=boot_d[:, :BOOT1W])
            nc.sync.dma_start(out=boot[:, BOOT1W:], in_=boot_d[:, BOOT1W:])
            wconst = boot[:, 0:WCW]

            blk = [None] * NPB
            for pb in range(1, NPB):
                w = BLKW if pb < NPB - 1 else NG * PW
                blk[pb] = bpool.tile([KC, w], F16, tag="blk", name=f"blk{pb}")
                src = AP(tensor=xrep_d, offset=(pb - 1) * BLKW,
                         ap=[[XREPW, KC], [1, w]])
                dst = AP(tensor=blk[pb].tensor, offset=blk[pb].offset,
                         ap=[[w, KC], [1, w]])
                nc.sync.dma_start(out=dst, in_=src)

            def stage1(pb):
                nr = NROW if pb < NPB - 1 else 1
                n = nr * W
                bw = BLKW if pb < NPB - 1 else NG * PW
                gw = bw // NG
                P = [psP.tile([128, NDEN], F32, tag="P", name=f"P{pb}_{m}")
                     for m in range(2)]
                for g in range(NG):
                    if pb == 0:
                        t, off, pitch = boot, boot.offset + WCW + g * NPIX, BOOTW
                    else:
                        t, off, pitch = blk[pb], blk[pb].offset + g * gw, bw
                    rhs = AP(tensor=t.tensor, offset=off,
                             ap=[[pitch, KC], [PW, nr], [1, W]])
                    for m in range(2):
                        lt = boot[0:KC, (g * 2 + m) * 128:(g * 2 + m + 1) * 128]
                        nc.tensor.matmul(P[m][:, :n], lt, rhs,
                                         start=(g == 0), stop=(g == NG - 1))
                A = [wpool.tile([128, NDEN], F16, tag="A", name=f"A{pb}_{m}")
                     for m in range(2)]
                # last (tiny) block: both copies on DVE — the scalar engine
                # is still busy with the previous block's copy
                nc.vector.tensor_copy(A[0][:, :n], P[0][:, :n])
                if pb == NPB - 1:
                    nc.vector.tensor_copy(A[1][:, :n], P[1][:, :n])
                else:
                    nc.scalar.copy(A[1][:, :n], P[1][:, :n])
                return A

            osb = None

            def stage2(pb, A):
                nonlocal osb
                nr = NROW if pb < NPB - 1 else 1
                n = nr * W
                R = psR.tile([32, NDEN], F32, tag="R", name=f"R{pb}")
                gm0 = wconst[:, 2 * NG * 128:2 * NG * 128 + 32]
                gm1 = wconst[:, 2 * NG * 128 + 32:2 * NG * 128 + 64]
                nc.tensor.matmul(R[:, :n], gm0, A[0][:, :n],
                                 start=True, stop=False)
                nc.tensor.matmul(R[:, :n], gm1, A[1][:, :n],
                                 start=False, stop=True)
                if pb % 2 == 0:
                    osb = opool.tile([32, 2 * NDEN], F32, tag="osb",
                                     name=f"osb{pb}")
                half = (pb % 2) * NDEN
                out_ap = AP(tensor=osb.tensor, offset=osb.offset + half,
                            ap=[[2 * NDEN, 32], [1, n]])
                # alternate osb copies between engines; keep the final pair's
                # copies off the congested scalar queue
                if pb % 2 == 0:
                    nc.vector.tensor_copy(out_ap, R[:, :n])
                else:
                    nc.scalar.copy(out_ap, R[:, :n])
                if pb % 2 == 1:
                    nrows = NROW + nr
                    dst = AP(tensor=out_d, offset=(pb - 1) * NDEN,
                             ap=[[H * W, OUT], [1, nrows * W]])
                    src = AP(tensor=osb.tensor, offset=osb.offset,
                             ap=[[2 * NDEN, 32], [1, nrows * W]])
                    nc.sync.dma_start(out=dst, in_=src)

            prevA = None
            for pb in range(NPB):
                A = stage1(pb)
                if prevA is not None:
                    stage2(pb - 1, prevA)
                prevA = A
            stage2(NPB - 1, prevA)

    _split_multiwaits(nc)
    return nc


def kernel(x, weight, bias):
    x = np.asarray(x, np.float32)
    weight = np.asarray(weight, np.float32)
    bias = np.asarray(bias, np.float32)

    xrep = _build_xrep(x)
    wc = _prep_weights(weight, bias)

    if "nc" not in _cache:
        _cache["nc"] = _build_nc()
    nc = _cache["nc"]

    in_maps = []
    for b in range(B):
        boot = np.zeros((128, BOOTW), np.float16)
        boot[:, :WCW] = wc
        boot[:KC, WCW:] = xrep[b, :, :BLKW]
        in_maps.append({"boot": boot, "xrep": xrep[b, :, BLKW:]})
    res = run_bass_kernel_spmd(nc, in_maps, core_ids=list(range(B)))
    out = np.stack([r["out"] for r in res.results])
    return out.reshape(B, OUT, H, W).astype(np.float32)
